# revision 10
# baseline (speedup 1.0000x reference)
"""Conv2D 3x3 (stride 1, pad 1) NCHW kernel for Trainium2, 8 NeuronCores.

Reference op: y = conv2d(x[32,128,56,56], w[256,128,3,3]) + b[256]  (fp32)

Strategy (MODE="wino"):
  - Data-parallel over batch: 4 images per core, 8 cores.
  - 1D Winograd F(2,3) along W: each output pair (y[2j], y[2j+1]) needs
    4 products instead of 6, cutting PE cycles by 1/3 vs direct conv.
      input transform  (DVE, fp16):  t0 = d0-d2, t1 = d1+d2,
                                     t2 = d2-d1, t3 = d1-d3
        with d_a = xpad[.., h, 2j+a]
      weight transform (host, fp32): k0 = g0, k1 = (g0+g1+g2)/2,
                                     k2 = (g0-g1+g2)/2, k3 = g2
      m_k[co, h, j]  = sum_ci sum_kh Wt[kh,k][ci,co] * t_k[ci, h+kh, j]
        (ci contracted on the PE partitions, kh accumulated in PSUM)
      inverse        (DVE+ACT):      y[2j] = m0+m1+m2 + bias
                                     y[2j+1] = m1-m2-m3 + bias
  - Per output block (8 rows x 56 cols): 12 matmuls of N=224 (=8x28
    pairs) vs direct's 9 of N=448: 2688 PE cycles vs 4032.
  - PSUM per group: two 1-bank tiles [128, 8, 28, 2] (k pairs split so
    no accumulation region crosses a PSUM bank).
  - x loads split across sync (priority head rows 0..9 of img0),
    scalar (rest of img0) and gpsimd (imgs 1-3) queues; transforms for
    img i+1 are interleaved between img i's groups so the DVE FIFO
    never blocks evictions.
"""

import numpy as np

N_CORES = 8
B, CI, H, W = 32, 128, 56, 56
CO = 256
KH = KW = 3
NTAPS = KH * KW
BS = B // N_CORES            # images per core
HP, WP = H + 2, W + 2        # padded image
HB = 8                       # output rows per block
NB = H // HB                 # blocks per image
NPAIR = W // 2               # winograd output pairs per row
NCHUNK = CO // 128           # co chunks of 128 partitions

MODE = "wino"                # "wino" | "direct"
WDTYPE = "float16"

_cache = {}


def _build_wino(reps=1, internal_io=False):
    import contextlib

    import concourse.mybir as mybir
    import concourse.tile as tile
    from concourse import bacc

    dt = mybir.dt
    nc = bacc.Bacc("TRN2", target_bir_lowering=False, debug=False)
    if internal_io:
        xp_ap = nc.dram_tensor("xp_i", [BS, CI, HP, WP], dt.float16).ap()
        y_ap = nc.dram_tensor("y_i", [BS, CO, H, W], dt.float32).ap()
        tok_ap = nc.dram_tensor(
            "tok", [128, NCHUNK], dt.float32, kind="ExternalOutput"
        ).ap()
    else:
        xp_ap = nc.dram_tensor(
            "xp", [BS, CI, HP, WP], dt.float16, kind="ExternalInput"
        ).ap()
        y_ap = nc.dram_tensor(
            "y", [BS, CO, H, W], dt.float32, kind="ExternalOutput"
        ).ap()
        tok_ap = None
    wt_ap = nc.dram_tensor(
        "wt", [CI, KH * 4 * CO], dt.float16, kind="ExternalInput"
    ).ap()
    bt_ap = nc.dram_tensor(
        "bt", [128, NCHUNK], dt.float32, kind="ExternalInput"
    ).ap()

    with tile.TileContext(nc) as tc:
        with (
            tc.tile_pool(name="xw", bufs=1) as xw,
            tc.tile_pool(name="tmp", bufs=12) as tmpp,
            tc.tile_pool(name="out", bufs=8) as outp,
            tc.tile_pool(name="ps", bufs=4, space="PSUM") as ps,
        ):
            wsb = xw.tile([CI, KH * 4 * CO], dt.float16, tag="w")
            bsb = xw.tile([128, NCHUNK], dt.float32, tag="b")
            nc.scalar.dma_start(out=wsb[:], in_=wt_ap[:, :])
            nc.scalar.dma_start(out=bsb[:], in_=bt_ap[:, :])
            wv = wsb[:].rearrange("c (kh k u o) -> c kh k u o", kh=KH, k=4, u=NCHUNK)

            loop_cm = (
                tc.For_i(0, reps, 1, hint_engines=(mybir.EngineType.PE,))
                if reps > 1
                else contextlib.nullcontext()
            )
            with loop_cm:
                xsb = xw.tile([CI, BS * HP * WP], dt.float16, tag="x")
                xtb = xw.tile([CI, BS * HP * NPAIR * 4], dt.float16, tag="xt")
                head = (HB + 2) * WP
                mid = 34 * WP
                xflat0 = xp_ap[0].rearrange("c h w -> c (h w)")
                nc.sync.dma_start(out=xsb[:, 0:head], in_=xflat0[:, 0:head])
                nc.scalar.dma_start(
                    out=xsb[:, head:mid], in_=xflat0[:, head:mid]
                )
                nc.gpsimd.dma_start(
                    out=xsb[:, mid : HP * WP], in_=xflat0[:, mid : HP * WP]
                )
                for img in range(1, BS):
                    nc.gpsimd.dma_start(
                        out=xsb[:, img * HP * WP : (img + 1) * HP * WP],
                        in_=xp_ap[img].rearrange("c h w -> c (h w)")[:, :],
                    )
                # pair view of padded input: [c, n, h, 29, 2]
                xr = xsb[:].rearrange(
                    "c (n h j p) -> c n h j p", n=BS, h=HP, j=WP // 2
                )
                # k OUTERMOST: each k-plane is contiguous so matmul rhs
                # streams at inner stride 1 (strided inner dims are slow)
                xt = xtb[:].rearrange(
                    "c (k n h j) -> c k n h j", k=4, n=BS, h=HP
                )

                def transform(img, r0, r1):
                    d0 = xr[:, img, r0:r1, 0:NPAIR, 0]
                    d1 = xr[:, img, r0:r1, 0:NPAIR, 1]
                    d2 = xr[:, img, r0:r1, 1 : NPAIR + 1, 0]
                    d3 = xr[:, img, r0:r1, 1 : NPAIR + 1, 1]
                    return [
                        ("sub", d0, d2, 0),
                        ("add", d1, d2, 1),
                        ("sub", d2, d1, 2),
                        ("sub", d1, d3, 3),
                    ], (img, r0, r1)

                def emit_t(op):
                    (kind, a, b, k), (img, r0, r1) = op
                    dst = xt[:, k, img, r0:r1, :]
                    if kind == "add":
                        nc.vector.tensor_add(dst, a, b)
                    else:
                        nc.vector.tensor_sub(dst, a, b)

                # img0 transform up-front, in chunks matching its three DMAs
                # so the first groups never wait on the whole image
                for r0, r1 in ((0, HB + 2), (HB + 2, 34), (34, HP)):
                    ops, meta = transform(0, r0, r1)
                    for op in ops:
                        emit_t((op, meta))

                pending = []
                for img in range(1, BS):
                    ops, meta = transform(img, 0, HP)
                    pending.extend((op, meta) for op in ops)

                gi = 0
                for img in range(BS):
                    for c in range(NCHUNK):
                        for hb in range(NB):
                            pt01 = ps.tile(
                                [128, 2, HB, NPAIR], dt.float32, tag="p01"
                            )
                            pt23 = ps.tile(
                                [128, 2, HB, NPAIR], dt.float32, tag="p23"
                            )
                            for k in range(4):
                                pdst = (pt01 if k < 2 else pt23)[:, k % 2]
                                for kh in range(KH):
                                    r0 = hb * HB + kh
                                    nc.tensor.matmul(
                                        pdst,
                                        wv[:, kh, k, c, :],
                                        xt[:, k, img, r0 : r0 + HB, :],
                                        start=(kh == 0),
                                        stop=(kh == KH - 1),
                                    )
                            m0 = pt01[:, 0]
                            m1 = pt01[:, 1]
                            m2 = pt23[:, 0]
                            m3 = pt23[:, 1]
                            # TensorTensor may read at most one PSUM operand:
                            # hoist m1 (+bias) to SBUF on ACT, then chain DVE
                            # ops with one PSUM input each.
                            #   y0 = m0+m1+m2+b = (c1+m0)+m2
                            #   y1 = m1-m2-m3+b = (c1-m2)-m3
                            c1 = tmpp.tile([128, HB, NPAIR], dt.float32, tag="c1")
                            s1 = tmpp.tile([128, HB, NPAIR], dt.float32, tag="s1")
                            t1 = tmpp.tile([128, HB, NPAIR], dt.float32, tag="t1")
                            nc.scalar.activation(
                                c1[:],
                                m1,
                                mybir.ActivationFunctionType.Identity,
                                bias=bsb[:, c : c + 1],
                                scale=1.0,
                            )
                            ot = outp.tile([128, HB * W], dt.float32, tag="o")
                            ov = ot[:].rearrange(
                                "c (h j q) -> c h j q", h=HB, j=NPAIR
                            )
                            nc.vector.tensor_add(s1[:], c1[:], m0)
                            nc.vector.tensor_add(ov[:, :, :, 0], s1[:], m2)
                            nc.vector.tensor_sub(t1[:], c1[:], m2)
                            nc.vector.tensor_sub(ov[:, :, :, 1], t1[:], m3)
                            nc.sync.dma_start(
                                out=y_ap[
                                    img,
                                    c * 128 : (c + 1) * 128,
                                    hb * HB : (hb + 1) * HB,
                                    :,
                                ],
                                in_=ot[:],
                            )
                            gi += 1
                            # feed next image's transform between groups
                            if pending and gi % 2 == 0:
                                emit_t(pending.pop(0))
                while pending:
                    emit_t(pending.pop(0))
            if tok_ap is not None:
                nc.sync.dma_start(out=tok_ap[:, :], in_=bsb[:])
    nc.compile()
    return nc


def _build_direct(reps=1, internal_io=False):
    """Fallback: direct conv as 9 shifted matmuls (the previous kernel)."""
    import contextlib

    import concourse.mybir as mybir
    import concourse.tile as tile
    from concourse import bacc

    mmdt = mybir.dt.float16
    nc = bacc.Bacc("TRN2", target_bir_lowering=False, debug=False)
    if internal_io:
        xp_ap = nc.dram_tensor("xp_i", [BS, CI, HP, WP], mmdt).ap()
        y_ap = nc.dram_tensor("y_i", [BS, CO, H, W], mybir.dt.float32).ap()
        tok_ap = nc.dram_tensor(
            "tok", [128, NCHUNK], mybir.dt.float32, kind="ExternalOutput"
        ).ap()
    else:
        xp_ap = nc.dram_tensor(
            "xp", [BS, CI, HP, WP], mmdt, kind="ExternalInput"
        ).ap()
        y_ap = nc.dram_tensor(
            "y", [BS, CO, H, W], mybir.dt.float32, kind="ExternalOutput"
        ).ap()
        tok_ap = None
    wt_ap = nc.dram_tensor(
        "wt", [CI, NTAPS * CO], mmdt, kind="ExternalInput"
    ).ap()
    bt_ap = nc.dram_tensor(
        "bt", [128, NCHUNK], mybir.dt.float32, kind="ExternalInput"
    ).ap()

    with tile.TileContext(nc) as tc:
        with (
            tc.tile_pool(name="xw", bufs=1) as xw,
            tc.tile_pool(name="out", bufs=8) as outp,
            tc.tile_pool(name="ps", bufs=7, space="PSUM") as ps,
        ):
            wsb = xw.tile([CI, NTAPS * CO], mmdt, tag="w")
            bsb = xw.tile([128, NCHUNK], mybir.dt.float32, tag="b")
            nc.scalar.dma_start(out=wsb[:], in_=wt_ap[:, :])
            nc.scalar.dma_start(out=bsb[:], in_=bt_ap[:, :])

            loop_cm = (
                tc.For_i(0, reps, 1, hint_engines=(mybir.EngineType.PE,))
                if reps > 1
                else contextlib.nullcontext()
            )
            with loop_cm:
                xsb = xw.tile([CI, BS * HP * WP], mmdt, tag="x")
                head = (HB + 2) * WP
                xflat0 = xp_ap[0].rearrange("c h w -> c (h w)")
                nc.sync.dma_start(out=xsb[:, 0:head], in_=xflat0[:, 0:head])
                nc.scalar.dma_start(
                    out=xsb[:, head : HP * WP], in_=xflat0[:, head : HP * WP]
                )
                for img in range(1, BS):
                    nc.gpsimd.dma_start(
                        out=xsb[:, img * HP * WP : (img + 1) * HP * WP],
                        in_=xp_ap[img].rearrange("c h w -> c (h w)")[:, :],
                    )
                xv = xsb[:].rearrange("c (n h w) -> c n h w", n=BS, h=HP)

                for c in range(NCHUNK):
                    for img in range(BS):
                        for hb in range(NB):
                            pt = ps.tile([128, HB * W], mybir.dt.float32, tag="acc")
                            for kh in range(KH):
                                for kw in range(KW):
                                    tap = kh * KW + kw
                                    r0 = hb * HB + kh
                                    nc.tensor.matmul(
                                        pt[:],
                                        wsb[
                                            :,
                                            tap * CO
                                            + c * 128 : tap * CO
                                            + (c + 1) * 128,
                                        ],
                                        xv[:, img, r0 : r0 + HB, kw : kw + W],
                                        start=(tap == 0),
                                        stop=(tap == NTAPS - 1),
                                    )
                            ot = outp.tile([128, HB * W], mybir.dt.float32, tag="o")
                            nc.scalar.activation(
                                ot[:],
                                pt[:],
                                mybir.ActivationFunctionType.Identity,
                                bias=bsb[:, c : c + 1],
                                scale=1.0,
                            )
                            nc.sync.dma_start(
                                out=y_ap[
                                    img,
                                    c * 128 : (c + 1) * 128,
                                    hb * HB : (hb + 1) * HB,
                                    :,
                                ],
                                in_=ot[:],
                            )
            if tok_ap is not None:
                nc.sync.dma_start(out=tok_ap[:, :], in_=bsb[:])
    nc.compile()
    return nc


def _get_nc(reps=1, mode=None, internal_io=False):
    mode = mode or MODE
    key = (reps, mode, internal_io)
    if key not in _cache:
        builder = _build_wino if mode == "wino" else _build_direct
        _cache[key] = builder(reps, internal_io)
    return _cache[key]


def _prep_inputs(x, weight, bias, mode=None):
    mode = mode or MODE
    x = np.asarray(x)
    weight = np.ascontiguousarray(weight, dtype=np.float32)
    bias = np.ascontiguousarray(bias, dtype=np.float32)
    xpad = np.zeros((B, CI, HP, WP), dtype=np.float16)
    xpad[:, :, 1 : H + 1, 1 : W + 1] = x
    if mode == "wino":
        g = weight  # [co, ci, kh, kw]
        k0 = g[..., 0]
        k1 = (g[..., 0] + g[..., 1] + g[..., 2]) * 0.5
        k2 = (g[..., 0] - g[..., 1] + g[..., 2]) * 0.5
        k3 = g[..., 2]
        kk = np.stack([k0, k1, k2, k3], axis=0)  # [4, co, ci, kh]
        # -> [ci, kh, k, co] -> [ci, kh, k, chunk, 128]
        wt = kk.transpose(2, 3, 0, 1).reshape(CI, KH * 4 * CO)
        wt = np.ascontiguousarray(wt.astype(np.float16))
    else:
        wt = np.ascontiguousarray(
            weight.transpose(1, 2, 3, 0).reshape(CI, NTAPS * CO).astype(np.float16)
        )
    bt = np.ascontiguousarray(bias.reshape(NCHUNK, 128).T)
    in_maps = [
        {
            "xp": np.ascontiguousarray(xpad[i * BS : (i + 1) * BS]),
            "wt": wt,
            "bt": bt,
        }
        for i in range(N_CORES)
    ]
    return in_maps


def timing_in_maps(mode=None):
    mode = mode or MODE
    rng = np.random.default_rng(0)
    cols = KH * 4 * CO if mode == "wino" else NTAPS * CO
    wt = rng.standard_normal((CI, cols)).astype(np.float16)
    bt = np.ascontiguousarray(
        rng.standard_normal((128, NCHUNK)).astype(np.float32)
    )
    return [{"wt": wt, "bt": bt} for _ in range(N_CORES)]


def run_sharded(x, weight, bias, trace=False, reps=1, mode=None):
    """Run on all 8 cores; returns (full_output, BassKernelResults)."""
    from concourse.bass_utils import run_bass_kernel_spmd

    mode = mode or MODE
    nc = _get_nc(reps, mode)
    in_maps = _prep_inputs(x, weight, bias, mode)
    res = run_bass_kernel_spmd(nc, in_maps, list(range(N_CORES)), trace=trace)
    y = np.concatenate([res.results[i]["y"] for i in range(N_CORES)], axis=0)
    return y, res


def kernel(x, weight, bias):
    y, _ = run_sharded(x, weight, bias)
    return y


# revision 12
# speedup vs baseline: 1.4165x; 1.4165x over previous
"""Conv2D 3x3 (stride 1, pad 1) NCHW kernel for Trainium2, 8 NeuronCores.

Reference op: y = conv2d(x[32,128,56,56], w[256,128,3,3]) + b[256]  (fp32)

Strategy (MODE="wino"):
  - Data-parallel over batch: 4 images per core, 8 cores.
  - 1D Winograd F(2,3) along W: each output pair (y[2j], y[2j+1]) needs
    4 products instead of 6, cutting PE cycles by 1/3 vs direct conv.
      input transform  (DVE, fp16):  t0 = d0-d2, t1 = d1+d2,
                                     t2 = d2-d1, t3 = d1-d3
        with d_a = xpad[.., h, 2j+a]
      weight transform (host, fp32): k0 = g0, k1 = (g0+g1+g2)/2,
                                     k2 = (g0-g1+g2)/2, k3 = g2
      m_k[co, h, j]  = sum_ci sum_kh Wt[kh,k][ci,co] * t_k[ci, h+kh, j]
        (ci contracted on the PE partitions, kh accumulated in PSUM)
      inverse        (DVE+ACT):      y[2j] = m0+m1+m2 + bias
                                     y[2j+1] = m1-m2-m3 + bias
  - Per output block (8 rows x 56 cols): 12 matmuls of N=224 (=8x28
    pairs) vs direct's 9 of N=448: 2688 PE cycles vs 4032.
  - PSUM per group: two 1-bank tiles [128, 8, 28, 2] (k pairs split so
    no accumulation region crosses a PSUM bank).
  - x loads split across sync (priority head rows 0..9 of img0),
    scalar (rest of img0) and gpsimd (imgs 1-3) queues; transforms for
    img i+1 are interleaved between img i's groups so the DVE FIFO
    never blocks evictions.
"""

import numpy as np

N_CORES = 8
B, CI, H, W = 32, 128, 56, 56
CO = 256
KH = KW = 3
NTAPS = KH * KW
BS = B // N_CORES            # images per core
HP, WP = H + 2, W + 2        # padded image
HB = 8                       # output rows per block
NB = H // HB                 # blocks per image
NPAIR = W // 2               # winograd output pairs per row
NCHUNK = CO // 128           # co chunks of 128 partitions

MODE = "wino"                # "wino" | "direct"
WDTYPE = "float16"

_cache = {}


def _build_wino(reps=1, internal_io=False):
    import contextlib

    import concourse.mybir as mybir
    import concourse.tile as tile
    from concourse import bacc

    dt = mybir.dt
    nc = bacc.Bacc("TRN2", target_bir_lowering=False, debug=False)
    if internal_io:
        xp_ap = nc.dram_tensor("xp_i", [BS, CI, HP, WP], dt.float16).ap()
        y_ap = nc.dram_tensor("y_i", [BS, CO, H, W], dt.float32).ap()
        tok_ap = nc.dram_tensor(
            "tok", [128, NCHUNK], dt.float32, kind="ExternalOutput"
        ).ap()
    else:
        xp_ap = nc.dram_tensor(
            "xp", [BS, CI, HP, WP], dt.float16, kind="ExternalInput"
        ).ap()
        y_ap = nc.dram_tensor(
            "y", [BS, CO, H, W], dt.float32, kind="ExternalOutput"
        ).ap()
        tok_ap = None
    wt_ap = nc.dram_tensor(
        "wt", [CI, KH * 4 * CO], dt.float16, kind="ExternalInput"
    ).ap()
    bt_ap = nc.dram_tensor(
        "bt", [128, NCHUNK], dt.float32, kind="ExternalInput"
    ).ap()

    with tile.TileContext(nc) as tc:
        with (
            tc.tile_pool(name="xw", bufs=1) as xw,
            tc.tile_pool(name="tmp", bufs=12) as tmpp,
            tc.tile_pool(name="out", bufs=8) as outp,
            tc.tile_pool(name="ps", bufs=4, space="PSUM") as ps,
        ):
            wsb = xw.tile([CI, KH * 4 * CO], dt.float16, tag="w")
            bsb = xw.tile([128, NCHUNK], dt.float32, tag="b")
            nc.scalar.dma_start(out=wsb[:], in_=wt_ap[:, :])
            nc.scalar.dma_start(out=bsb[:], in_=bt_ap[:, :])
            wv = wsb[:].rearrange("c (kh k u o) -> c kh k u o", kh=KH, k=4, u=NCHUNK)

            loop_cm = (
                tc.For_i(0, reps, 1, hint_engines=(mybir.EngineType.PE,))
                if reps > 1
                else contextlib.nullcontext()
            )
            with loop_cm:
                xsb = xw.tile([CI, BS * HP * WP], dt.float16, tag="x")
                xtb = xw.tile([CI, BS * HP * NPAIR * 4], dt.float16, tag="xt")
                head = (HB + 2) * WP
                mid = 34 * WP
                xflat0 = xp_ap[0].rearrange("c h w -> c (h w)")
                nc.sync.dma_start(out=xsb[:, 0:head], in_=xflat0[:, 0:head])
                nc.scalar.dma_start(
                    out=xsb[:, head:mid], in_=xflat0[:, head:mid]
                )
                nc.gpsimd.dma_start(
                    out=xsb[:, mid : HP * WP], in_=xflat0[:, mid : HP * WP]
                )
                for img in range(1, BS):
                    nc.gpsimd.dma_start(
                        out=xsb[:, img * HP * WP : (img + 1) * HP * WP],
                        in_=xp_ap[img].rearrange("c h w -> c (h w)")[:, :],
                    )
                # pair view of padded input: [c, n, h, 29, 2]
                xr = xsb[:].rearrange(
                    "c (n h j p) -> c n h j p", n=BS, h=HP, j=WP // 2
                )
                # k OUTERMOST: each k-plane is contiguous so matmul rhs
                # streams at inner stride 1 (strided inner dims are slow)
                xt = xtb[:].rearrange(
                    "c (k n h j) -> c k n h j", k=4, n=BS, h=HP
                )

                def transform(img, r0, r1):
                    d0 = xr[:, img, r0:r1, 0:NPAIR, 0]
                    d1 = xr[:, img, r0:r1, 0:NPAIR, 1]
                    d2 = xr[:, img, r0:r1, 1 : NPAIR + 1, 0]
                    d3 = xr[:, img, r0:r1, 1 : NPAIR + 1, 1]
                    return [
                        ("sub", d0, d2, 0),
                        ("add", d1, d2, 1),
                        ("sub", d2, d1, 2),
                        ("sub", d1, d3, 3),
                    ], (img, r0, r1)

                def emit_t(op, eng=None):
                    (kind, a, b, k), (img, r0, r1) = op
                    eng = eng or nc.vector
                    dst = xt[:, k, img, r0:r1, :]
                    if kind == "add":
                        eng.tensor_add(dst, a, b)
                    else:
                        eng.tensor_sub(dst, a, b)

                # img0 transform up-front, in chunks matching its three DMAs
                # so the first groups never wait on the whole image
                for r0, r1 in ((0, HB + 2), (HB + 2, 34), (34, HP)):
                    ops, meta = transform(0, r0, r1)
                    for op in ops:
                        emit_t((op, meta))

                # imgs 1-3: transforms on the otherwise-idle GpSimd engine
                # (DVE is co-critical with the PE: ~224 eviction ops/iter).
                # Their loads land early and the results aren't needed until
                # group 14/28/42, so Q7 latency is immaterial.
                for img in range(1, BS):
                    ops, meta = transform(img, 0, HP)
                    for op in ops:
                        emit_t((op, meta), eng=nc.gpsimd)
                pending = []

                gi = 0
                for img in range(BS):
                    for c in range(NCHUNK):
                        for hb in range(NB):
                            pt01 = ps.tile(
                                [128, 2, HB, NPAIR], dt.float32, tag="p01"
                            )
                            pt23 = ps.tile(
                                [128, 2, HB, NPAIR], dt.float32, tag="p23"
                            )
                            for k in range(4):
                                pdst = (pt01 if k < 2 else pt23)[:, k % 2]
                                for kh in range(KH):
                                    r0 = hb * HB + kh
                                    nc.tensor.matmul(
                                        pdst,
                                        wv[:, kh, k, c, :],
                                        xt[:, k, img, r0 : r0 + HB, :],
                                        start=(kh == 0),
                                        stop=(kh == KH - 1),
                                    )
                            m0 = pt01[:, 0]
                            m1 = pt01[:, 1]
                            m2 = pt23[:, 0]
                            m3 = pt23[:, 1]
                            # TensorTensor may read at most one PSUM operand:
                            # hoist m1 (+bias) to SBUF on ACT, then chain DVE
                            # ops with one PSUM input each.
                            #   y0 = m0+m1+m2+b = (c1+m0)+m2
                            #   y1 = m1-m2-m3+b = (c1-m2)-m3
                            c1 = tmpp.tile([128, HB, NPAIR], dt.float32, tag="c1")
                            s1 = tmpp.tile([128, HB, NPAIR], dt.float32, tag="s1")
                            t1 = tmpp.tile([128, HB, NPAIR], dt.float32, tag="t1")
                            nc.scalar.activation(
                                c1[:],
                                m1,
                                mybir.ActivationFunctionType.Identity,
                                bias=bsb[:, c : c + 1],
                                scale=1.0,
                            )
                            ot = outp.tile([128, HB * W], dt.float32, tag="o")
                            ov = ot[:].rearrange(
                                "c (h j q) -> c h j q", h=HB, j=NPAIR
                            )
                            nc.vector.tensor_add(s1[:], c1[:], m0)
                            nc.vector.tensor_add(ov[:, :, :, 0], s1[:], m2)
                            nc.vector.tensor_sub(t1[:], c1[:], m2)
                            nc.vector.tensor_sub(ov[:, :, :, 1], t1[:], m3)
                            nc.sync.dma_start(
                                out=y_ap[
                                    img,
                                    c * 128 : (c + 1) * 128,
                                    hb * HB : (hb + 1) * HB,
                                    :,
                                ],
                                in_=ot[:],
                            )
                            gi += 1
                            # feed next image's transform between groups
                            if pending and gi % 2 == 0:
                                emit_t(pending.pop(0))
                while pending:
                    emit_t(pending.pop(0))
            if tok_ap is not None:
                nc.sync.dma_start(out=tok_ap[:, :], in_=bsb[:])
    nc.compile()
    return nc


def _build_direct(reps=1, internal_io=False):
    """Fallback: direct conv as 9 shifted matmuls (the previous kernel)."""
    import contextlib

    import concourse.mybir as mybir
    import concourse.tile as tile
    from concourse import bacc

    mmdt = mybir.dt.float16
    nc = bacc.Bacc("TRN2", target_bir_lowering=False, debug=False)
    if internal_io:
        xp_ap = nc.dram_tensor("xp_i", [BS, CI, HP, WP], mmdt).ap()
        y_ap = nc.dram_tensor("y_i", [BS, CO, H, W], mybir.dt.float32).ap()
        tok_ap = nc.dram_tensor(
            "tok", [128, NCHUNK], mybir.dt.float32, kind="ExternalOutput"
        ).ap()
    else:
        xp_ap = nc.dram_tensor(
            "xp", [BS, CI, HP, WP], mmdt, kind="ExternalInput"
        ).ap()
        y_ap = nc.dram_tensor(
            "y", [BS, CO, H, W], mybir.dt.float32, kind="ExternalOutput"
        ).ap()
        tok_ap = None
    wt_ap = nc.dram_tensor(
        "wt", [CI, NTAPS * CO], mmdt, kind="ExternalInput"
    ).ap()
    bt_ap = nc.dram_tensor(
        "bt", [128, NCHUNK], mybir.dt.float32, kind="ExternalInput"
    ).ap()

    with tile.TileContext(nc) as tc:
        with (
            tc.tile_pool(name="xw", bufs=1) as xw,
            tc.tile_pool(name="out", bufs=8) as outp,
            tc.tile_pool(name="ps", bufs=7, space="PSUM") as ps,
        ):
            wsb = xw.tile([CI, NTAPS * CO], mmdt, tag="w")
            bsb = xw.tile([128, NCHUNK], mybir.dt.float32, tag="b")
            nc.scalar.dma_start(out=wsb[:], in_=wt_ap[:, :])
            nc.scalar.dma_start(out=bsb[:], in_=bt_ap[:, :])

            loop_cm = (
                tc.For_i(0, reps, 1, hint_engines=(mybir.EngineType.PE,))
                if reps > 1
                else contextlib.nullcontext()
            )
            with loop_cm:
                xsb = xw.tile([CI, BS * HP * WP], mmdt, tag="x")
                head = (HB + 2) * WP
                xflat0 = xp_ap[0].rearrange("c h w -> c (h w)")
                nc.sync.dma_start(out=xsb[:, 0:head], in_=xflat0[:, 0:head])
                nc.scalar.dma_start(
                    out=xsb[:, head : HP * WP], in_=xflat0[:, head : HP * WP]
                )
                for img in range(1, BS):
                    nc.gpsimd.dma_start(
                        out=xsb[:, img * HP * WP : (img + 1) * HP * WP],
                        in_=xp_ap[img].rearrange("c h w -> c (h w)")[:, :],
                    )
                xv = xsb[:].rearrange("c (n h w) -> c n h w", n=BS, h=HP)

                for c in range(NCHUNK):
                    for img in range(BS):
                        for hb in range(NB):
                            pt = ps.tile([128, HB * W], mybir.dt.float32, tag="acc")
                            for kh in range(KH):
                                for kw in range(KW):
                                    tap = kh * KW + kw
                                    r0 = hb * HB + kh
                                    nc.tensor.matmul(
                                        pt[:],
                                        wsb[
                                            :,
                                            tap * CO
                                            + c * 128 : tap * CO
                                            + (c + 1) * 128,
                                        ],
                                        xv[:, img, r0 : r0 + HB, kw : kw + W],
                                        start=(tap == 0),
                                        stop=(tap == NTAPS - 1),
                                    )
                            ot = outp.tile([128, HB * W], mybir.dt.float32, tag="o")
                            nc.scalar.activation(
                                ot[:],
                                pt[:],
                                mybir.ActivationFunctionType.Identity,
                                bias=bsb[:, c : c + 1],
                                scale=1.0,
                            )
                            nc.sync.dma_start(
                                out=y_ap[
                                    img,
                                    c * 128 : (c + 1) * 128,
                                    hb * HB : (hb + 1) * HB,
                                    :,
                                ],
                                in_=ot[:],
                            )
            if tok_ap is not None:
                nc.sync.dma_start(out=tok_ap[:, :], in_=bsb[:])
    nc.compile()
    return nc


def _get_nc(reps=1, mode=None, internal_io=False):
    mode = mode or MODE
    key = (reps, mode, internal_io)
    if key not in _cache:
        builder = _build_wino if mode == "wino" else _build_direct
        _cache[key] = builder(reps, internal_io)
    return _cache[key]


def _prep_inputs(x, weight, bias, mode=None):
    mode = mode or MODE
    x = np.asarray(x)
    weight = np.ascontiguousarray(weight, dtype=np.float32)
    bias = np.ascontiguousarray(bias, dtype=np.float32)
    xpad = np.zeros((B, CI, HP, WP), dtype=np.float16)
    xpad[:, :, 1 : H + 1, 1 : W + 1] = x
    if mode == "wino":
        g = weight  # [co, ci, kh, kw]
        k0 = g[..., 0]
        k1 = (g[..., 0] + g[..., 1] + g[..., 2]) * 0.5
        k2 = (g[..., 0] - g[..., 1] + g[..., 2]) * 0.5
        k3 = g[..., 2]
        kk = np.stack([k0, k1, k2, k3], axis=0)  # [4, co, ci, kh]
        # -> [ci, kh, k, co] -> [ci, kh, k, chunk, 128]
        wt = kk.transpose(2, 3, 0, 1).reshape(CI, KH * 4 * CO)
        wt = np.ascontiguousarray(wt.astype(np.float16))
    else:
        wt = np.ascontiguousarray(
            weight.transpose(1, 2, 3, 0).reshape(CI, NTAPS * CO).astype(np.float16)
        )
    bt = np.ascontiguousarray(bias.reshape(NCHUNK, 128).T)
    in_maps = [
        {
            "xp": np.ascontiguousarray(xpad[i * BS : (i + 1) * BS]),
            "wt": wt,
            "bt": bt,
        }
        for i in range(N_CORES)
    ]
    return in_maps


def timing_in_maps(mode=None):
    mode = mode or MODE
    rng = np.random.default_rng(0)
    cols = KH * 4 * CO if mode == "wino" else NTAPS * CO
    wt = rng.standard_normal((CI, cols)).astype(np.float16)
    bt = np.ascontiguousarray(
        rng.standard_normal((128, NCHUNK)).astype(np.float32)
    )
    return [{"wt": wt, "bt": bt} for _ in range(N_CORES)]


def run_sharded(x, weight, bias, trace=False, reps=1, mode=None):
    """Run on all 8 cores; returns (full_output, BassKernelResults)."""
    from concourse.bass_utils import run_bass_kernel_spmd

    mode = mode or MODE
    nc = _get_nc(reps, mode)
    in_maps = _prep_inputs(x, weight, bias, mode)
    res = run_bass_kernel_spmd(nc, in_maps, list(range(N_CORES)), trace=trace)
    y = np.concatenate([res.results[i]["y"] for i in range(N_CORES)], axis=0)
    return y, res


def kernel(x, weight, bias):
    y, _ = run_sharded(x, weight, bias)
    return y
